# revision 14
# baseline (speedup 1.0000x reference)
"""Trainium2 Bass kernel for nn_Classifier (EmbeddingBag-mean + label attention).

Data-parallel over 8 NeuronCores: each core handles 8 of the 64 batch items;
the embedding table (cast to bf16 on host) and the small class/multi params
are replicated.

Per core pipeline:
  1. dma_gather (SWDGE gather, int16 indices) of token embedding rows.
     The 100K vocab exceeds int16, so the table is processed as 4 chunks of
     25000 rows; the host buckets each batch-item's 8192 (sentence, token)
     pairs by chunk, sorts by sentence, pads each bucket to a fixed CAP, and
     bakes the wrapped int16 index streams. Gathered rows land round-robin
     across partitions: stream position i -> partition i%128, slot i//128.

     Gathers are issued as prepare_only descriptor-generation + trigger_dma
     so the GpSimd engine queue is not held for the DMA duration: many
     gathers stream concurrently across the 4 SWDGE queues. Pad slots use
     index -1 (trailing) with the true bucket count passed through a
     register, so pads generate no DMA descriptors at all. The first 8
     buckets per core gather at full CAP (pad index 0) to initialize every
     gather-pool buffer with finite data; afterwards stale pad rows are
     multiplied by an all-zero selection column, contributing 0.
  2. Reduction rows->sentences on the PE: for each 128-row block, a
     host-described selection matrix sel[p, s] = (sid[p] == s) (built on-chip
     by a DVE is_equal against a [128,128] s-ramp broadcast over blocks) maps
     gathered rows to sentence accumulators, accumulated across blocks in
     PSUM (f32).
  3. Per batch: PE transposes + matmuls for class-attention scores, ACT
     softmax (exp with accumulated sum), PE mix matmul, DVE dot with
     multi_weight, final 1/(L*sumexp) scale + bias.

The 1/L mean factor is folded into the host-prepared class_embs.T (for the
scores) and into the final per-class normalization (for the logits), so the
gathered sums are used raw.
"""

import numpy as np

import concourse.bass as bass
import concourse.tile as tile
from concourse import bacc, mybir
from concourse.bass_utils import run_bass_kernel_spmd

try:
    import ml_dtypes

    BF16 = np.float16
except ImportError:  # pragma: no cover
    BF16 = None

# Problem shapes (hardcoded; kernel.py must be self-contained).
V, E, C = 100000, 256, 100
B, S, L = 64, 128, 64
NCORES = 8
BSH = B // NCORES       # batch items (= sentence groups) per core
NCH = 4                 # vocab chunks (int16 index limit)
CHUNK = V // NCH        # 25000 rows per chunk
CAP = 2176              # padded bucket size; seed-0 max is 2145
NBUF = 8                # gather/sel pool depth (must divide first-full rule)
NOSENT = 200.0          # sid pad value, never equals a sentence id

_cache: dict = {}


def _build(cap: int = CAP) -> bacc.Bacc:
    key = ("nc", cap)
    if key in _cache:
        return _cache[key]

    blk = cap // 128
    cols = cap // 16

    nc = bacc.Bacc(
        "TRN2",
        target_bir_lowering=False,
        debug=False,
        num_devices=NCORES,
        num_swdge_queues=4,
        dynamic_dma_scratch_size=65536,
    )
    f32 = mybir.dt.float32
    bf16 = mybir.dt.float16
    i16 = mybir.dt.int16
    i32 = mybir.dt.int32

    emb_d = nc.dram_tensor("emb", [V, E], bf16, kind="ExternalInput").ap()
    idx_d = nc.dram_tensor("idx", [S, BSH * NCH * cols], i16, kind="ExternalInput").ap()
    cnt_d = nc.dram_tensor("cnt", [1, BSH * NCH], i32, kind="ExternalInput").ap()
    sid_d = nc.dram_tensor("sid", [S, BSH * NCH * blk], bf16, kind="ExternalInput").ap()
    srg_d = nc.dram_tensor("srg", [S, S], bf16, kind="ExternalInput").ap()
    cet_d = nc.dram_tensor("cet", [128, 2 * C], f32, kind="ExternalInput").ap()
    mw_d = nc.dram_tensor("mw", [C, E], f32, kind="ExternalInput").ap()
    mb_d = nc.dram_tensor("mb", [C, 1], f32, kind="ExternalInput").ap()
    idn_d = nc.dram_tensor("idn", [128, 128], f32, kind="ExternalInput").ap()
    logt_d = nc.dram_tensor("logt", [C, BSH], f32, kind="ExternalOutput").ap()

    AX = mybir.AxisListType
    OP = mybir.AluOpType
    AF = mybir.ActivationFunctionType

    with tile.TileContext(nc) as tc:
        with (
            tc.tile_pool(name="const", bufs=1) as cpool,
            tc.tile_pool(name="gather", bufs=NBUF) as gpool,
            tc.tile_pool(name="sel", bufs=NBUF) as selpool,
            tc.tile_pool(name="sents", bufs=3) as spool,
            tc.tile_pool(name="attn", bufs=2) as apool,
            tc.tile_pool(name="psacc", bufs=2, space="PSUM") as ppool,
            tc.tile_pool(name="psattn", bufs=1, space="PSUM") as qpool,
        ):
            # gather-critical inputs first so the first prep can issue early
            cnt = cpool.tile([1, BSH * NCH], i32)
            nc.sync.dma_start(out=cnt[:], in_=cnt_d[:])
            idx = cpool.tile([S, BSH * NCH * cols], i16)
            nc.sync.dma_start(out=idx[:], in_=idx_d[:])
            sid = cpool.tile([S, BSH * NCH * blk], bf16)
            nc.sync.dma_start(out=sid[:], in_=sid_d[:])
            srg = cpool.tile([S, S], bf16)
            nc.sync.dma_start(out=srg[:], in_=srg_d[:])
            cet = cpool.tile([128, 2 * C], f32)
            nc.sync.dma_start(out=cet[:], in_=cet_d[:])
            mw = cpool.tile([C, E], f32)
            nc.sync.dma_start(out=mw[:], in_=mw_d[:])
            mb = cpool.tile([C, 1], f32)
            nc.sync.dma_start(out=mb[:], in_=mb_d[:])
            ident = cpool.tile([128, 128], f32)
            nc.sync.dma_start(out=ident[:], in_=idn_d[:])
            logt = cpool.tile([C, BSH], f32)

            # Explicit sync for the prepare_only gather pipeline: gsems[q]
            # counts DMA completions on queue q (+16 each); pesems[q] counts
            # PE bucket consumption. Tile's automatic gating does not cover
            # gen_mode==1 preps, so RAW (matmul vs gather landing) and WAR
            # (re-trigger vs PE reads of the recycled buffer) are hand-wired.
            gsems = [nc.alloc_semaphore(f"gsem{i}") for i in range(BSH * NCH)]
            for s in gsems:
                nc.gpsimd.sem_clear(s)
            nregs = [nc.gpsimd.alloc_register(f"nidx{q}") for q in range(NCH)]

            for g in range(BSH):
                # --- phase A: gather + selection-matmul token-sum
                acc = ppool.tile([S, E], f32, tag="acc")
                for c in range(NCH):
                    gc = g * NCH + c
                    G = gpool.tile([S, blk * E], bf16, tag="G")
                    nc.gpsimd.reg_load(nregs[c], cnt[0:1, gc : gc + 1])
                    nc.gpsimd.dma_gather(
                        out_ap=G[:].rearrange("p (k e) -> p k e", e=E),
                        in_ap=emb_d[c * CHUNK : (c + 1) * CHUNK, :],
                        idxs_ap=idx[:, gc * cols : (gc + 1) * cols],
                        num_idxs=cap,
                        num_idxs_reg=nregs[c],
                        elem_size=E,
                        single_packet=False,
                        queue_num=c,
                        prepare_only=True,
                        sem=gsems[gc],
                    )
                    nc.gpsimd.trigger_dma(count=None, queue_num=c)
                    # sel[p, k*128+s] = (sid[p, gc*blk+k] == s), bf16 0/1
                    sel = selpool.tile([S, blk * S], bf16, tag="sel")
                    sid_sl = sid[:, gc * blk : (gc + 1) * blk]
                    sid_bc = bass.AP(
                        sid_sl.tensor,
                        sid_sl.offset,
                        [sid_sl.ap[0], sid_sl.ap[1], [0, S]],
                    )
                    srg_sl = srg[:]
                    srg_bc = bass.AP(
                        srg_sl.tensor,
                        srg_sl.offset,
                        [srg_sl.ap[0], [0, blk], srg_sl.ap[1]],
                    )
                    nc.vector.tensor_tensor(
                        out=sel[:].rearrange("p (k s) -> p k s", s=S),
                        in0=sid_bc,
                        in1=srg_bc,
                        op=OP.is_equal,
                    )
                    for j in range(blk):
                        mm = nc.tensor.matmul(
                            out=acc[:],
                            lhsT=sel[:, j * S : (j + 1) * S],
                            rhs=G[:, j * E : (j + 1) * E],
                            start=(c == 0 and j == 0),
                            stop=(c == NCH - 1 and j == blk - 1),
                        )
                        if j == 0:
                            mm._wait_ge(gsems[gc], 16)
                sents = spool.tile([S, E], f32, tag="sents")
                nc.vector.tensor_copy(out=sents[:], in_=acc[:])

                # --- phase B: attention for this batch item
                stj = []
                for j in range(2):
                    tp = qpool.tile([128, 128], f32, tag="tp")
                    nc.tensor.transpose(
                        out=tp[:], in_=sents[:, j * 128 : (j + 1) * 128], identity=ident[:]
                    )
                    st = apool.tile([128, 128], f32, tag=f"st{j}")
                    nc.vector.tensor_copy(out=st[:], in_=tp[:])
                    stj.append(st)
                scores = qpool.tile([C, S], f32, tag="scores")
                for j in range(2):
                    nc.tensor.matmul(
                        out=scores[:],
                        lhsT=cet[:, j * C : (j + 1) * C],
                        rhs=stj[j][:],
                        start=(j == 0),
                        stop=(j == 1),
                    )
                negmax = apool.tile([C, 1], f32, tag="negmax")
                nc.vector.tensor_reduce(
                    out=negmax[:], in_=scores[:], axis=AX.X, op=OP.max, negate=True
                )
                exps = apool.tile([C, S], f32, tag="exps")
                sume = apool.tile([C, 1], f32, tag="sume")
                nc.scalar.activation(
                    out=exps[:], in_=scores[:], func=AF.Exp, bias=negmax[:], accum_out=sume[:]
                )
                etp = qpool.tile([S, C], f32, tag="etp")
                nc.tensor.transpose(out=etp[:], in_=exps[:], identity=ident[0:C, 0:C])
                expsT = apool.tile([S, C], f32, tag="expsT")
                nc.vector.tensor_copy(out=expsT[:], in_=etp[:])
                mix = qpool.tile([C, E], f32, tag="mix")
                nc.tensor.matmul(out=mix[:], lhsT=expsT[:], rhs=sents[:], start=True, stop=True)
                prod = apool.tile([C, E], f32, tag="prod")
                red = apool.tile([C, 1], f32, tag="red")
                nc.vector.tensor_tensor(
                    out=prod[:], in0=mix[:], in1=mw[:], op=OP.mult
                )
                nc.vector.tensor_reduce(
                    out=red[:], in_=prod[:], axis=AX.X, op=OP.add
                )
                d64 = apool.tile([C, 1], f32, tag="d64")
                nc.vector.tensor_scalar_mul(d64[:], sume[:], float(L))
                rcp = apool.tile([C, 1], f32, tag="rcp")
                nc.vector.reciprocal(out=rcp[:], in_=d64[:])
                nc.vector.tensor_scalar(
                    out=logt[:, g : g + 1],
                    in0=red[:],
                    scalar1=rcp[:],
                    scalar2=mb[:],
                    op0=OP.mult,
                    op1=OP.add,
                )

            nc.sync.dma_start(out=logt_d[:], in_=logt[:])

    nc.compile()
    _cache[key] = nc
    return nc


def _host_prep(inputs: dict, cap: int = CAP):
    tok = np.asarray(inputs["tok_lists_batch"])
    emb = np.asarray(inputs["emb_weight"], dtype=np.float32)
    ce = np.asarray(inputs["class_embs"], dtype=np.float32)
    mwt = np.ascontiguousarray(np.asarray(inputs["multi_weight"], dtype=np.float32))
    mbs = np.ascontiguousarray(
        np.asarray(inputs["multi_bias"], dtype=np.float32).reshape(C, 1)
    )

    blk = cap // 128
    cols = cap // 16

    emb_bf = np.ascontiguousarray(emb.astype(BF16))

    # cet[p, j*C + c] = class_embs[c, j*128 + p] / L
    cet = (ce.T / np.float32(L)).astype(np.float32)  # [256, 100]
    cet = np.ascontiguousarray(
        cet.reshape(2, 128, C).transpose(1, 0, 2).reshape(128, 2 * C)
    )

    srg = np.ascontiguousarray(
        np.broadcast_to(np.arange(S).astype(BF16), (S, S))
    )
    idn = np.eye(128, dtype=np.float32)

    in_maps = []
    max_n = 0
    for core in range(NCORES):
        idx_all = np.zeros((S, BSH * NCH * cols), dtype=np.int16)
        sid_all = np.full((S, BSH * NCH * blk), NOSENT, dtype=BF16)
        cnt_all = np.zeros((1, BSH * NCH), dtype=np.int32)
        for g in range(BSH):
            t = np.asarray(tok[core * BSH + g], dtype=np.int64)  # [128, 64]
            chunk_of = t // CHUNK
            for c in range(NCH):
                ss, ll = np.nonzero(chunk_of == c)  # row-major: sorted by sentence
                n = len(ss)
                max_n = max(max_n, n)
                if n > cap:
                    return None, max_n  # caller rebuilds with bigger cap
                gc = g * NCH + c
                # First NBUF buckets gather at full cap (pad idx 0) so every
                # gather-pool buffer starts with finite data; later buckets
                # pad with -1 (no DMA descriptors) and pass the true count.
                first_round = gc < NBUF
                pad_idx = 0 if first_round else -1
                idx_stream = np.full(cap, pad_idx, dtype=np.int16)
                idx_stream[:n] = (t[ss, ll] - c * CHUNK).astype(np.int16)
                cnt_all[0, gc] = cap if first_round else max(n, 1)
                if not first_round and n == 0:  # keep >=1 valid index
                    idx_stream[0] = 0
                sid_stream = np.full(cap, NOSENT, dtype=BF16)
                sid_stream[:n] = ss.astype(BF16)
                idx_all[:, gc * cols : (gc + 1) * cols] = np.tile(
                    idx_stream.reshape(cols, 16).T, (8, 1)
                )
                sid_all[:, gc * blk : (gc + 1) * blk] = sid_stream.reshape(blk, S).T
        in_maps.append(
            {
                "emb": emb_bf,
                "idx": np.ascontiguousarray(idx_all),
                "cnt": cnt_all,
                "sid": np.ascontiguousarray(sid_all),
                "srg": srg,
                "cet": cet,
                "mw": mwt,
                "mb": mbs,
                "idn": idn,
            }
        )
    return in_maps, max_n


def run(inputs: dict, **kwargs):
    cap = CAP
    in_maps, max_n = _host_prep(inputs, cap)
    while in_maps is None:  # astronomically unlikely; rebuild with bigger cap
        cap = ((max_n + 127) // 128 + 1) * 128
        in_maps, max_n = _host_prep(inputs, cap)
    nc = _build(cap)
    res = run_bass_kernel_spmd(nc, in_maps, core_ids=list(range(NCORES)), **kwargs)
    out = np.empty((B, C), dtype=np.float32)
    for core in range(NCORES):
        out[core * BSH : (core + 1) * BSH] = res.results[core]["logt"].T
    return out, res


def kernel(**inputs) -> np.ndarray:
    out, _ = run(inputs)
    return out


# revision 18
# speedup vs baseline: 3.3716x; 3.3716x over previous
"""Trainium2 Bass kernel for nn_Classifier (EmbeddingBag-mean + label attention).

Data-parallel over 8 NeuronCores: each core handles 8 of the 64 batch items;
the embedding table (cast to bf16 on host) and the small class/multi params
are replicated.

Per core pipeline:
  1. dma_gather (SWDGE gather, int16 indices) of token embedding rows.
     The 100K vocab exceeds int16, so the table is processed as 4 chunks of
     25000 rows; the host buckets each batch-item's 8192 (sentence, token)
     pairs by chunk, sorts by sentence, pads each bucket to a fixed CAP, and
     bakes the wrapped int16 index streams. Gathered rows land round-robin
     across partitions: stream position i -> partition i%128, slot i//128.

     Gathers are issued as prepare_only descriptor-generation + trigger_dma
     so the GpSimd engine queue is not held for the DMA duration: many
     gathers stream concurrently across the 4 SWDGE queues. Pad slots use
     index -1 (trailing) with the true bucket count passed through a
     register, so pads generate no DMA descriptors at all. The first 8
     buckets per core gather at full CAP (pad index 0) to initialize every
     gather-pool buffer with finite data; afterwards stale pad rows are
     multiplied by an all-zero selection column, contributing 0.
  2. Reduction rows->sentences on the PE: for each 128-row block, a
     host-described selection matrix sel[p, s] = (sid[p] == s) (built on-chip
     by a DVE is_equal against a [128,128] s-ramp broadcast over blocks) maps
     gathered rows to sentence accumulators, accumulated across blocks in
     PSUM (f32).
  3. Per batch: PE transposes + matmuls for class-attention scores, ACT
     softmax (exp with accumulated sum), PE mix matmul, DVE dot with
     multi_weight, final 1/(L*sumexp) scale + bias.

The 1/L mean factor is folded into the host-prepared class_embs.T (for the
scores) and into the final per-class normalization (for the logits), so the
gathered sums are used raw.
"""

import numpy as np

import concourse.bass as bass
import concourse.tile as tile
from concourse import bacc, mybir
from concourse.bass_utils import run_bass_kernel_spmd

try:
    import ml_dtypes

    BF16 = np.float16
except ImportError:  # pragma: no cover
    BF16 = None

# Problem shapes (hardcoded; kernel.py must be self-contained).
V, E, C = 100000, 256, 100
B, S, L = 64, 128, 64
NCORES = 8
BSH = B // NCORES       # batch items (= sentence groups) per core
NCH = 4                 # vocab chunks (int16 index limit)
CHUNK = V // NCH        # 25000 rows per chunk
CAP = 2176              # padded bucket size; seed-0 max is 2145
NBUF = 8                # gather/sel pool depth (must divide first-full rule)
NOSENT = 200.0          # sid pad value, never equals a sentence id

_cache: dict = {}


def _build(cap: int = CAP) -> bacc.Bacc:
    key = ("nc", cap)
    if key in _cache:
        return _cache[key]

    blk = cap // 128
    cols = cap // 16

    nc = bacc.Bacc(
        "TRN2",
        target_bir_lowering=False,
        debug=False,
        num_devices=NCORES,
        num_swdge_queues=4,
    )
    f32 = mybir.dt.float32
    bf16 = mybir.dt.float16
    i16 = mybir.dt.int16
    i32 = mybir.dt.int32

    emb_d = nc.dram_tensor("emb", [V, E], bf16, kind="ExternalInput").ap()
    idx_d = nc.dram_tensor("idx", [S, BSH * NCH * cols], i16, kind="ExternalInput").ap()
    cnt_d = nc.dram_tensor("cnt", [1, BSH * NCH], i32, kind="ExternalInput").ap()
    sid_d = nc.dram_tensor("sid", [S, BSH * NCH * blk], bf16, kind="ExternalInput").ap()
    srg_d = nc.dram_tensor("srg", [S, S], bf16, kind="ExternalInput").ap()
    cet_d = nc.dram_tensor("cet", [128, 2 * C], f32, kind="ExternalInput").ap()
    mw_d = nc.dram_tensor("mw", [C, E], f32, kind="ExternalInput").ap()
    mb_d = nc.dram_tensor("mb", [C, 1], f32, kind="ExternalInput").ap()
    idn_d = nc.dram_tensor("idn", [128, 128], f32, kind="ExternalInput").ap()
    logt_d = nc.dram_tensor("logt", [C, BSH], f32, kind="ExternalOutput").ap()

    AX = mybir.AxisListType
    OP = mybir.AluOpType
    AF = mybir.ActivationFunctionType

    with tile.TileContext(nc) as tc:
        with (
            tc.tile_pool(name="const", bufs=1) as cpool,
            tc.tile_pool(name="gather", bufs=NBUF) as gpool,
            tc.tile_pool(name="sel", bufs=NBUF) as selpool,
            tc.tile_pool(name="sents", bufs=3) as spool,
            tc.tile_pool(name="attn", bufs=2) as apool,
            tc.tile_pool(name="psacc", bufs=2, space="PSUM") as ppool,
            tc.tile_pool(name="psattn", bufs=1, space="PSUM") as qpool,
        ):
            # gather-critical inputs first so the first prep can issue early
            cnt = cpool.tile([1, BSH * NCH], i32)
            nc.sync.dma_start(out=cnt[:], in_=cnt_d[:])
            idx = cpool.tile([S, BSH * NCH * cols], i16)
            nc.sync.dma_start(out=idx[:], in_=idx_d[:])
            sid = cpool.tile([S, BSH * NCH * blk], bf16)
            nc.sync.dma_start(out=sid[:], in_=sid_d[:])
            srg = cpool.tile([S, S], bf16)
            nc.sync.dma_start(out=srg[:], in_=srg_d[:])
            cet = cpool.tile([128, 2 * C], f32)
            nc.sync.dma_start(out=cet[:], in_=cet_d[:])
            mw = cpool.tile([C, E], f32)
            nc.sync.dma_start(out=mw[:], in_=mw_d[:])
            mb = cpool.tile([C, 1], f32)
            nc.sync.dma_start(out=mb[:], in_=mb_d[:])
            ident = cpool.tile([128, 128], f32)
            nc.sync.dma_start(out=ident[:], in_=idn_d[:])
            logt = cpool.tile([C, BSH], f32)

            # All 32 per-bucket counts go into Pool registers upfront via one
            # multi-register load, so the gather stream has no per-gather aux
            # instructions on the Pool engine: 32 bare dma_gathers fill the
            # 4-deep exec queue and keep all 4 SWDGE queues streaming.
            nregs = [
                nc.gpsimd.alloc_register(f"nidx{i}") for i in range(BSH * NCH)
            ]
            nc.gpsimd.reg_load(nregs, cnt[0:1, 0 : BSH * NCH])

            for g in range(BSH):
                # --- phase A: gather + selection-matmul token-sum
                acc = ppool.tile([S, E], f32, tag="acc")
                for c in range(NCH):
                    gc = g * NCH + c
                    G = gpool.tile([S, blk * E], bf16, tag="G")
                    nc.gpsimd.dma_gather(
                        out_ap=G[:].rearrange("p (k e) -> p k e", e=E),
                        in_ap=emb_d[c * CHUNK : (c + 1) * CHUNK, :],
                        idxs_ap=idx[:, gc * cols : (gc + 1) * cols],
                        num_idxs=cap,
                        num_idxs_reg=nregs[gc],
                        elem_size=E,
                        single_packet=False,
                        queue_num=c,
                    )
                    # sel[p, k*128+s] = (sid[p, gc*blk+k] == s), bf16 0/1
                    sel = selpool.tile([S, blk * S], bf16, tag="sel")
                    sid_sl = sid[:, gc * blk : (gc + 1) * blk]
                    sid_bc = bass.AP(
                        sid_sl.tensor,
                        sid_sl.offset,
                        [sid_sl.ap[0], sid_sl.ap[1], [0, S]],
                    )
                    srg_sl = srg[:]
                    srg_bc = bass.AP(
                        srg_sl.tensor,
                        srg_sl.offset,
                        [srg_sl.ap[0], [0, blk], srg_sl.ap[1]],
                    )
                    nc.vector.tensor_tensor(
                        out=sel[:].rearrange("p (k s) -> p k s", s=S),
                        in0=sid_bc,
                        in1=srg_bc,
                        op=OP.is_equal,
                    )
                    for j in range(blk):
                        nc.tensor.matmul(
                            out=acc[:],
                            lhsT=sel[:, j * S : (j + 1) * S],
                            rhs=G[:, j * E : (j + 1) * E],
                            start=(c == 0 and j == 0),
                            stop=(c == NCH - 1 and j == blk - 1),
                        )
                sents = spool.tile([S, E], f32, tag="sents")
                nc.vector.tensor_copy(out=sents[:], in_=acc[:])

                # --- phase B: attention for this batch item
                stj = []
                for j in range(2):
                    tp = qpool.tile([128, 128], f32, tag="tp")
                    nc.tensor.transpose(
                        out=tp[:], in_=sents[:, j * 128 : (j + 1) * 128], identity=ident[:]
                    )
                    st = apool.tile([128, 128], f32, tag=f"st{j}")
                    nc.vector.tensor_copy(out=st[:], in_=tp[:])
                    stj.append(st)
                scores = qpool.tile([C, S], f32, tag="scores")
                for j in range(2):
                    nc.tensor.matmul(
                        out=scores[:],
                        lhsT=cet[:, j * C : (j + 1) * C],
                        rhs=stj[j][:],
                        start=(j == 0),
                        stop=(j == 1),
                    )
                negmax = apool.tile([C, 1], f32, tag="negmax")
                nc.vector.tensor_reduce(
                    out=negmax[:], in_=scores[:], axis=AX.X, op=OP.max, negate=True
                )
                exps = apool.tile([C, S], f32, tag="exps")
                sume = apool.tile([C, 1], f32, tag="sume")
                nc.scalar.activation(
                    out=exps[:], in_=scores[:], func=AF.Exp, bias=negmax[:], accum_out=sume[:]
                )
                etp = qpool.tile([S, C], f32, tag="etp")
                nc.tensor.transpose(out=etp[:], in_=exps[:], identity=ident[0:C, 0:C])
                expsT = apool.tile([S, C], f32, tag="expsT")
                nc.vector.tensor_copy(out=expsT[:], in_=etp[:])
                mix = qpool.tile([C, E], f32, tag="mix")
                nc.tensor.matmul(out=mix[:], lhsT=expsT[:], rhs=sents[:], start=True, stop=True)
                prod = apool.tile([C, E], f32, tag="prod")
                red = apool.tile([C, 1], f32, tag="red")
                nc.vector.tensor_tensor(
                    out=prod[:], in0=mix[:], in1=mw[:], op=OP.mult
                )
                nc.vector.tensor_reduce(
                    out=red[:], in_=prod[:], axis=AX.X, op=OP.add
                )
                d64 = apool.tile([C, 1], f32, tag="d64")
                nc.vector.tensor_scalar_mul(d64[:], sume[:], float(L))
                rcp = apool.tile([C, 1], f32, tag="rcp")
                nc.vector.reciprocal(out=rcp[:], in_=d64[:])
                nc.vector.tensor_scalar(
                    out=logt[:, g : g + 1],
                    in0=red[:],
                    scalar1=rcp[:],
                    scalar2=mb[:],
                    op0=OP.mult,
                    op1=OP.add,
                )

            nc.sync.dma_start(out=logt_d[:], in_=logt[:])

    nc.compile()
    _cache[key] = nc
    return nc


def _host_prep(inputs: dict, cap: int = CAP):
    tok = np.asarray(inputs["tok_lists_batch"])
    emb = np.asarray(inputs["emb_weight"], dtype=np.float32)
    ce = np.asarray(inputs["class_embs"], dtype=np.float32)
    mwt = np.ascontiguousarray(np.asarray(inputs["multi_weight"], dtype=np.float32))
    mbs = np.ascontiguousarray(
        np.asarray(inputs["multi_bias"], dtype=np.float32).reshape(C, 1)
    )

    blk = cap // 128
    cols = cap // 16

    emb_bf = np.ascontiguousarray(emb.astype(BF16))

    # cet[p, j*C + c] = class_embs[c, j*128 + p] / L
    cet = (ce.T / np.float32(L)).astype(np.float32)  # [256, 100]
    cet = np.ascontiguousarray(
        cet.reshape(2, 128, C).transpose(1, 0, 2).reshape(128, 2 * C)
    )

    srg = np.ascontiguousarray(
        np.broadcast_to(np.arange(S).astype(BF16), (S, S))
    )
    idn = np.eye(128, dtype=np.float32)

    in_maps = []
    max_n = 0
    for core in range(NCORES):
        idx_all = np.zeros((S, BSH * NCH * cols), dtype=np.int16)
        sid_all = np.full((S, BSH * NCH * blk), NOSENT, dtype=BF16)
        cnt_all = np.zeros((1, BSH * NCH), dtype=np.int32)
        for g in range(BSH):
            t = np.asarray(tok[core * BSH + g], dtype=np.int64)  # [128, 64]
            chunk_of = t // CHUNK
            for c in range(NCH):
                ss, ll = np.nonzero(chunk_of == c)  # row-major: sorted by sentence
                n = len(ss)
                max_n = max(max_n, n)
                if n > cap:
                    return None, max_n  # caller rebuilds with bigger cap
                gc = g * NCH + c
                # First NBUF buckets gather at full cap (pad idx 0) so every
                # gather-pool buffer starts with finite data; later buckets
                # pad with -1 (no DMA descriptors) and pass the true count.
                first_round = gc < NBUF
                pad_idx = 0 if first_round else -1
                idx_stream = np.full(cap, pad_idx, dtype=np.int16)
                idx_stream[:n] = (t[ss, ll] - c * CHUNK).astype(np.int16)
                cnt_all[0, gc] = cap if first_round else max(n, 1)
                if not first_round and n == 0:  # keep >=1 valid index
                    idx_stream[0] = 0
                sid_stream = np.full(cap, NOSENT, dtype=BF16)
                sid_stream[:n] = ss.astype(BF16)
                idx_all[:, gc * cols : (gc + 1) * cols] = np.tile(
                    idx_stream.reshape(cols, 16).T, (8, 1)
                )
                sid_all[:, gc * blk : (gc + 1) * blk] = sid_stream.reshape(blk, S).T
        in_maps.append(
            {
                "emb": emb_bf,
                "idx": np.ascontiguousarray(idx_all),
                "cnt": cnt_all,
                "sid": np.ascontiguousarray(sid_all),
                "srg": srg,
                "cet": cet,
                "mw": mwt,
                "mb": mbs,
                "idn": idn,
            }
        )
    return in_maps, max_n


def run(inputs: dict, **kwargs):
    cap = CAP
    in_maps, max_n = _host_prep(inputs, cap)
    while in_maps is None:  # astronomically unlikely; rebuild with bigger cap
        cap = ((max_n + 127) // 128 + 1) * 128
        in_maps, max_n = _host_prep(inputs, cap)
    nc = _build(cap)
    res = run_bass_kernel_spmd(nc, in_maps, core_ids=list(range(NCORES)), **kwargs)
    out = np.empty((B, C), dtype=np.float32)
    for core in range(NCORES):
        out[core * BSH : (core + 1) * BSH] = res.results[core]["logt"].T
    return out, res


def kernel(**inputs) -> np.ndarray:
    out, _ = run(inputs)
    return out
